# revision 16
# baseline (speedup 1.0000x reference)
"""MoE-LoRA layer kernel for Trainium2 (8 NeuronCores, data-parallel over tokens).

Computation (per reference):
  out = x @ W_base.T + b_base + scaling * sum_e combine[:,e] * (x @ A_e.T) @ B_e.T
  combine = renormalized top-2 softmax of router logits (= softmax over top-2 logits).

Sharding: 8192 tokens -> 1024 per core; all weights replicated. Everything
is laid out host-side so device DMAs are contiguous:
  xt[p, kt, t]      = x[t, kt*128+p]           (x transposed, k-tiled)
  wt[ot, p, kt, o]  = W_base[ot*128+o, kt*128+p]  (W_base.T per-o-tile slabs)
  at[p, kt, er]     = A_all[er, kt*128+p]
  bt[er, o]         = B_stack[e, o, r],  er = e*16+r
  rt[p, kt, e]      = W_router[e, kt*128+p]
  bias2[p, ot]      = b_base[ot*128+p]
Output: outt[ot, p, t] = out[t, ot*128+p].

All matmuls run as float32r (full fp32 input bits; measured 227 ns/matmul
at 512-wide vs bf16's 259 ns -- f32r streams faster on this hardware).
The softmax/top-2 combine chain is interleaved into the ot=1/ot=2 base
matmul streams (per-kt hooks) so the PE never waits on the DVE, and each
o-tile's B-matmul/bias/store tail is deferred until after the next
o-tile's base matmuls for the same reason.
"""

import sys
import numpy as np
from contextlib import ExitStack

try:
    import concourse.bass as bass
except ImportError:
    sys.path.insert(0, "/opt/trn_rl_repo")
    import concourse.bass as bass

import ml_dtypes
import concourse.tile as tile
from concourse import bacc
from concourse import mybir
from concourse.bass import ts
from concourse.bass_utils import run_bass_kernel_spmd

F32 = mybir.dt.float32
F32R = mybir.dt.float32r
BF16 = mybir.dt.bfloat16
ALU = mybir.AluOpType
ACTF = mybir.ActivationFunctionType
AX = mybir.AxisListType
NPBF = ml_dtypes.bfloat16

N_CORES = 8
D_IN = 4096
D_OUT = 4096
RANK = 16
NUM_EXPERTS = 8
ER = NUM_EXPERTS * RANK  # 128
TOP_K = 2
SCALING = 32.0 / RANK  # 2.0


def build_nc(T=1024, KT=32, OT=32):
    """Build the per-core Bass kernel. T tokens, KT k-tiles (d_in=128*KT),
    OT out-tiles (d_out=128*OT). T must be a multiple of 512."""
    TH = T // 512  # token halves for 512-wide matmuls
    TS = T // 128  # token subtiles for router/softmax
    nc = bacc.Bacc(None, target_bir_lowering=False, dynamic_dma_scratch_size=1024)

    xt = nc.dram_tensor("xt", [128, KT, T], F32R, kind="ExternalInput")
    wt = nc.dram_tensor("wt", [OT, 128, KT, 128], F32R, kind="ExternalInput")
    at = nc.dram_tensor("at", [128, KT, ER], F32R, kind="ExternalInput")
    bt = nc.dram_tensor("bt", [ER, 128 * OT], F32R, kind="ExternalInput")
    rt = nc.dram_tensor("rt", [128, KT, NUM_EXPERTS], F32R, kind="ExternalInput")
    bias2 = nc.dram_tensor("bias2", [128, OT], F32, kind="ExternalInput")
    id2 = nc.dram_tensor("id2", [128, 128], F32, kind="ExternalInput")
    expand = nc.dram_tensor("expand", [NUM_EXPERTS, ER], F32, kind="ExternalInput")
    outt = nc.dram_tensor("outt", [OT, 128, T], F32, kind="ExternalOutput")

    with tile.TileContext(nc) as tc, ExitStack() as ctx:
        const = ctx.enter_context(tc.tile_pool(name="const", bufs=1))
        xpool = ctx.enter_context(tc.tile_pool(name="xp", bufs=1))
        wpool = ctx.enter_context(tc.tile_pool(name="wp", bufs=4))
        btp = ctx.enter_context(tc.tile_pool(name="btp", bufs=2))
        hpool = ctx.enter_context(tc.tile_pool(name="hp", bufs=1))
        smt = ctx.enter_context(tc.tile_pool(name="smt", bufs=2))
        opool = ctx.enter_context(tc.tile_pool(name="op", bufs=4))
        pmain = ctx.enter_context(
            tc.tile_pool(name="pmain", bufs=max(3 * TH, 4), space="PSUM")
        )
        psmall = ctx.enter_context(tc.tile_pool(name="psm", bufs=2, space="PSUM"))
        E = NUM_EXPERTS

        # ---- window DMAs (sync HWDGE queue), front-loaded so kt=0 lands
        # fast; bf16 halves every transfer vs the fp32 version ----
        at_s = wpool.tile([128, KT, 128], F32R, tag="w")
        w0_s = wpool.tile([128, KT, 128], F32R, tag="w")
        rt_s = const.tile([128, KT, E], F32R)
        x_s = xpool.tile([128, KT, T], F32R)
        nsw = min(8, KT)
        wpc = KT // nsw

        id_s = bias_s = exp_s = b0_s = None

        def emit_consts():
            nonlocal id_s, bias_s, exp_s, b0_s
            id_s = const.tile([128, 128], F32)
            nc.scalar.dma_start(id_s, id2[:])
            bias_s = const.tile([128, OT], F32)
            nc.scalar.dma_start(bias_s, bias2[:])
            exp_s = const.tile([E, ER], F32)
            nc.scalar.dma_start(exp_s, expand[:])
            b0_s = const.tile([ER, 128], F32R)
            nc.scalar.dma_start(b0_s, bt[:, 0:128])

        def xdma(kt, eng):
            if kt < 2 and KT >= 32:
                for th in range(TH):
                    eng.dma_start(x_s[:, kt, ts(th, 512)], xt[:, kt, ts(th, 512)])
            else:
                eng.dma_start(x_s[:, kt, :], xt[:, kt, :])

        # front-loaded window DMAs, interleaved so the first k-tiles land
        # fast (baseline-tuned pattern: th-split head, 1MB x chunks after)
        for q in range(nsw):
            if q == 0 and wpc > 1:
                nc.scalar.dma_start(at_s[:, 0:1, :], at[:, 0:1, :])
                nc.scalar.dma_start(rt_s, rt[:])
                nc.scalar.dma_start(w0_s[:, 0:1, :], wt[0, :, 0:1, :])
                if KT >= 32:
                    for th in range(TH):
                        nc.sync.dma_start(
                            x_s[:, 0, ts(th, 512)], xt[:, 0, ts(th, 512)]
                        )
                nc.scalar.dma_start(at_s[:, 1:wpc, :], at[:, 1:wpc, :])
                nc.scalar.dma_start(w0_s[:, 1:wpc, :], wt[0, :, 1:wpc, :])
            else:
                nc.scalar.dma_start(at_s[:, ts(q, wpc), :], at[:, ts(q, wpc), :])
                nc.scalar.dma_start(w0_s[:, ts(q, wpc), :], wt[0, :, ts(q, wpc), :])
            if q == 0 or KT < 32:
                for kt in range(q * wpc, (q + 1) * wpc):
                    if q == 0 and kt == 0 and wpc > 1 and KT >= 32:
                        continue
                    xdma(kt, nc.sync)
            else:
                # 2-ktile (1MB) chunks: fewer descriptors, higher sustained rate
                for k0 in range(q * wpc, (q + 1) * wpc, 2):
                    nc.sync.dma_start(x_s[:, k0 : k0 + 2, :], xt[:, k0 : k0 + 2, :])
            if q == 5:
                emit_consts()
        if id_s is None:
            emit_consts()

        # ---- window: A-proj + router + base(ot=0) share the x stream ----
        ph = [pmain.tile([128, 512], F32, tag="pm", name=f"ph{i}") for i in range(TH)]
        plT = [pmain.tile([E, 512], F32, tag="pm", name=f"plT{i}") for i in range(TH)]
        po0 = [pmain.tile([128, 512], F32, tag="pm", name=f"po0{i}") for i in range(TH)]
        for kt in range(KT):
            st, sp = kt == 0, kt == KT - 1
            xcs = [x_s[:, kt, ts(th, 512)] for th in range(TH)]
            if kt < 2:
                # th-major: start on the first 512 tokens before the
                # second half of the kt tile has landed
                for th in range(TH):
                    nc.tensor.matmul(ph[th], at_s[:, kt, :], xcs[th], start=st, stop=sp)
                    nc.tensor.matmul(plT[th], rt_s[:, kt, :], xcs[th], start=st, stop=sp)
                    nc.tensor.matmul(po0[th], w0_s[:, kt, :], xcs[th], start=st, stop=False)
            else:
                for th in range(TH):
                    nc.tensor.matmul(ph[th], at_s[:, kt, :], xcs[th], start=st, stop=sp)
                for th in range(TH):
                    nc.tensor.matmul(plT[th], rt_s[:, kt, :], xcs[th], start=st, stop=sp)
                for th in range(TH):
                    nc.tensor.matmul(po0[th], w0_s[:, kt, :], xcs[th], start=st, stop=False)

        def load_w(ot):
            w_s = wpool.tile([128, KT, 128], F32R, tag="w")
            nsl = min(4, KT)
            for q in range(nsl):
                nc.scalar.dma_start(
                    w_s[:, ts(q, KT // nsl), :], wt[ot, :, ts(q, KT // nsl), :]
                )
            b_sl = btp.tile([ER, 128], F32R)
            nc.scalar.dma_start(b_sl, bt[:, ts(ot, 128)])
            return w_s, b_sl

        h_s = hpool.tile([128, T], F32R)  # A-proj, then weighted in place
        h_f = h_s.bitcast(F32)  # f32 view for DVE ops
        lT = hpool.tile([E, T], F32)
        # lT first: the ptl transposes interleaved into base(1) wait on it
        for th in range(TH):
            nc.vector.tensor_copy(lT[:, ts(th, 512)], plT[th])
        for th in range(TH):
            nc.vector.tensor_copy(h_s[:, ts(th, 512)], ph[th])

        # ---- softmax/top-2 chain, emitted piecewise between base matmuls ----
        ptl_all = psmall.tile([128, E * TS], F32, tag="ptl", bufs=1)
        cmb2s = [None] * TS
        e_ts = [None] * TS
        e12s = [None] * TS
        invs = [None] * TS
        cts = [None] * TS

        def sm_pre(s_i):
            # PE: transpose logits subtile to [128 tok, E]; all TS subtiles
            # share one PSUM bank (disjoint columns) to avoid ring waits
            nc.tensor.transpose(
                ptl_all[:, ts(s_i, E)], lT[:, ts(s_i, 128)], id_s[:E, :E]
            )

        def sm_dve_head(s_i):
            # DVE: maxes + top-2 mask; ACT: exps (no max-shift, |logit| < ~4)
            l = ptl_all[:, ts(s_i, E)]
            m12 = smt.tile([128, 2], F32, tag="m12", name=f"m12_{s_i}")
            nc.vector.reduce_max(m12[:, 0:1], l, axis=AX.X)
            pen = smt.tile([128, E], F32, tag="pen", name=f"pen{s_i}")
            nc.vector.tensor_scalar(
                pen, l, m12[:, 0:1], -1e30, op0=ALU.is_equal, op1=ALU.mult
            )
            msk = smt.tile([128, E], F32, tag="msk", name=f"msk{s_i}")
            nc.vector.tensor_tensor(msk, l, pen, op=ALU.add)
            nc.vector.reduce_max(m12[:, 1:2], msk, axis=AX.X)
            ge = smt.tile([128, E], F32, tag="ge", bufs=4, name=f"ge{s_i}")
            nc.vector.tensor_scalar(ge, l, m12[:, 1:2], None, op0=ALU.is_ge)
            e12 = smt.tile([128, 2], F32, tag="e12", bufs=4, name=f"e12_{s_i}")
            nc.scalar.activation(e12, m12, ACTF.Exp)
            e_t = smt.tile([128, E], F32, tag="e_t", bufs=4, name=f"e_t{s_i}")
            nc.scalar.activation(e_t, l, ACTF.Exp)
            e12s[s_i] = e12
            e_ts[s_i] = (e_t, ge)

        def sm_dve_tail(s_i):
            # DVE: renormalize (runs 2 hooks later so the ACT exps are done)
            e12 = e12s[s_i]
            den = smt.tile([128, 1], F32, tag="den", name=f"den{s_i}")
            nc.vector.tensor_tensor(den, e12[:, 0:1], e12[:, 1:2], op=ALU.add)
            inv = smt.tile([128, 1], F32, tag="inv", bufs=4, name=f"inv{s_i}")
            nc.vector.reciprocal(inv, den)
            e_t, ge = e_ts[s_i]
            cmb = smt.tile([128, E], F32, tag="cmb", name=f"cmb{s_i}")
            nc.vector.tensor_tensor(cmb, e_t, ge, op=ALU.mult)
            # all TS cmb2 tiles are alive until their (late) pt transpose;
            # fewer bufs deadlocks the DVE queue against the PE queue
            cmb2 = smt.tile([128, E], F32, tag="cmb2", bufs=TS, name=f"cmb2{s_i}")
            nc.vector.tensor_scalar(cmb2, cmb, inv, None, op0=ALU.mult)
            cmb2s[s_i] = cmb2

        def sm_pt(s_i):
            # PE: transpose combine back to [E, 128 tok]; DVE: copy out
            p = psmall.tile([E, 128], F32, tag="ps", bufs=1, name=f"pt{s_i}")
            nc.tensor.transpose(p, cmb2s[s_i], id_s)
            ct = smt.tile([E, 128], F32, tag="ct", name=f"ct{s_i}")
            nc.vector.tensor_copy(ct, p)
            cts[s_i] = ct

        def sm_pc(s_i):
            # PE: expand combine to [ER, 128] (x scaling); DVE: weight h
            p = psmall.tile([128, 128], F32, tag="ps", bufs=1, name=f"pc{s_i}")
            nc.tensor.matmul(p, exp_s, cts[s_i], start=True, stop=True)
            nc.vector.tensor_tensor(
                h_s[:, ts(s_i, 128)], h_f[:, ts(s_i, 128)], p, op=ALU.mult
            )

        def emit_base(ot, w_s, hooks=None):
            # kt outer / th inner: consecutive matmuls share the stationary
            # weight tile; optional per-kt hooks inject softmax-chain ops
            # into the PE stream so they hide under the matmuls
            pos = [
                pmain.tile([128, 512], F32, tag="pm", name=f"po_{ot}_{th}")
                for th in range(TH)
            ]
            for kt in range(KT):
                for th in range(TH):
                    nc.tensor.matmul(
                        pos[th],
                        w_s[:, kt, :],
                        x_s[:, kt, ts(th, 512)],
                        start=(kt == 0),
                        stop=False,
                    )
                if hooks is not None:
                    for fn in hooks.get(kt, ()):
                        fn()
            return pos

        def emit_tail(ot, pos, b_sl):
            for th in range(TH):
                nc.tensor.matmul(
                    pos[th], b_sl, h_s[:, ts(th, 512)], start=False, stop=True
                )
                o_t = opool.tile([128, 512], F32, tag="o_t", name=f"ot_{ot}_{th}")
                nc.scalar.activation(
                    o_t, pos[th], ACTF.Identity, bias=bias_s[:, ot : ot + 1]
                )
                nc.sync.dma_start(outt[ot, :, ts(th, 512)], o_t)

        # hook schedules: ptl transposes early (paced for the 2-slot psmall
        # ring), combine transposes/expands late; overflow into base(2)
        hooks1 = {}
        for s in range(TS):
            hooks1.setdefault(2 + 2 * s, []).append(
                lambda s=s: (sm_pre(s), sm_dve_head(s))
            )
            hooks1.setdefault(6 + 2 * s, []).append(lambda s=s: sm_dve_tail(s))
        hooks2 = {}
        for s in range(TS):
            hooks2.setdefault(2 * s, []).append(lambda s=s: sm_pt(s))
            hooks2.setdefault(2 * s + 1, []).append(lambda s=s: sm_pc(s))

        first = min(1, OT - 1)
        w1, b1 = load_w(first)
        w2, b2 = load_w(2)
        pos1 = emit_base(first, w1, hooks1)
        pos2 = emit_base(2, w2, hooks2)

        emit_tail(first, pos1, b1)

        # ---- ot=0 LoRA term accumulated into the held PSUM group ----
        for th in range(TH):
            nc.tensor.matmul(
                po0[th], b0_s, h_s[:, ts(th, 512)], start=False, stop=True
            )
            o_t = opool.tile([128, 512], F32, name=f"oo0_{th}", tag="o_t")
            nc.scalar.activation(o_t, po0[th], ACTF.Identity, bias=bias_s[:, 0:1])
            nc.sync.dma_start(outt[0, :, ts(th, 512)], o_t)

        # ---- remaining o-tiles; tail(ot-1) after base(ot) so the B matmul
        # never waits on the combine chain and output DMA stays 1-ot behind
        prev = (2, pos2, b2)
        for ot in range(3, OT):
            w_s, b_sl = load_w(ot)
            pos = emit_base(ot, w_s)
            emit_tail(*prev)
            prev = (ot, pos, b_sl)
        emit_tail(*prev)

    nc.compile()
    return nc


def prep_shared(W_base, b_base, W_router, A_stack, B_stack, KT=32, OT=32):
    """Host-side layout prep for the replicated weights (bf16)."""
    D = KT * 128
    O = OT * 128
    W_base = np.asarray(W_base, dtype=np.float32)
    wt = np.ascontiguousarray(
        W_base.reshape(OT, 128, KT, 128).transpose(0, 3, 2, 1)
    )
    A_all = np.asarray(A_stack, dtype=np.float32).reshape(ER, D)
    at = np.ascontiguousarray(A_all.reshape(ER, KT, 128).transpose(2, 1, 0))
    bt = np.ascontiguousarray(
        np.asarray(B_stack, dtype=np.float32).transpose(0, 2, 1).reshape(ER, O)
    )
    rtT = np.asarray(W_router, dtype=np.float32).T  # [D, E]
    rt = np.ascontiguousarray(rtT.reshape(KT, 128, NUM_EXPERTS).transpose(1, 0, 2))
    bias2 = np.ascontiguousarray(np.asarray(b_base, dtype=np.float32).reshape(OT, 128).T)
    id2 = np.eye(128, dtype=np.float32)
    expand = np.repeat(
        np.eye(NUM_EXPERTS, dtype=np.float32) * np.float32(SCALING), RANK, axis=1
    )
    return dict(wt=wt, at=at, bt=bt, rt=rt, bias2=bias2, id2=id2, expand=expand)


def make_in_maps(x, W_base, b_base, W_router, A_stack, B_stack, KT=32, OT=32):
    x = np.asarray(x, dtype=np.float32)
    xf = x.reshape(-1, D_IN)
    N = xf.shape[0]
    T = N // N_CORES
    shared = prep_shared(W_base, b_base, W_router, A_stack, B_stack, KT, OT)
    in_maps = []
    for c in range(N_CORES):
        x_c = xf[c * T : (c + 1) * T]  # [T, D]
        xtc = np.ascontiguousarray(x_c.reshape(T, KT, 128).transpose(2, 1, 0))
        m = dict(shared)
        m["xt"] = xtc
        in_maps.append(m)
    return in_maps


_NC_CACHE = {}


def _get_nc(T, KT, OT):
    key = (T, KT, OT)
    if key not in _NC_CACHE:
        _NC_CACHE[key] = build_nc(T, KT, OT)
    return _NC_CACHE[key]


def kernel(x, W_base, b_base, W_router, A_stack, B_stack):
    x = np.asarray(x, dtype=np.float32)
    orig_shape = x.shape
    N = x.reshape(-1, D_IN).shape[0]
    T = N // N_CORES
    KT = D_IN // 128
    OT = D_OUT // 128

    nc = _get_nc(T, KT, OT)
    in_maps = make_in_maps(x, W_base, b_base, W_router, A_stack, B_stack, KT, OT)

    res = run_bass_kernel_spmd(nc, in_maps, core_ids=list(range(N_CORES)))
    out = np.empty((N, D_OUT), dtype=np.float32)
    for c in range(N_CORES):
        outt = res.results[c]["outt"]  # [OT, 128, T]
        out[c * T : (c + 1) * T] = outt.transpose(2, 0, 1).reshape(T, D_OUT)
    return out.reshape(orig_shape[:-1] + (D_OUT,))


# revision 17
# speedup vs baseline: 1.0219x; 1.0219x over previous
"""MoE-LoRA layer kernel for Trainium2 (8 NeuronCores, data-parallel over tokens).

Computation (per reference):
  out = x @ W_base.T + b_base + scaling * sum_e combine[:,e] * (x @ A_e.T) @ B_e.T
  combine = renormalized top-2 softmax of router logits (= softmax over top-2 logits).

Sharding: 8192 tokens -> 1024 per core; all weights replicated. Everything
is laid out host-side so device DMAs are contiguous:
  xt[p, kt, t]      = x[t, kt*128+p]           (x transposed, k-tiled)
  wt[ot, p, kt, o]  = W_base[ot*128+o, kt*128+p]  (W_base.T per-o-tile slabs)
  at[p, kt, er]     = A_all[er, kt*128+p]
  bt[er, o]         = B_stack[e, o, r],  er = e*16+r
  rt[p, kt, e]      = W_router[e, kt*128+p]
  bias2[p, ot]      = b_base[ot*128+p]
Output: outt[ot, p, t] = out[t, ot*128+p].

All matmuls run as float32r (full fp32 input bits; measured 227 ns/matmul
at 512-wide vs bf16's 259 ns -- f32r streams faster on this hardware).
The softmax/top-2 combine chain is interleaved into the ot=1/ot=2 base
matmul streams (per-kt hooks) so the PE never waits on the DVE, and each
o-tile's B-matmul/bias/store tail is deferred until after the next
o-tile's base matmuls for the same reason.
"""

import sys
import numpy as np
from contextlib import ExitStack

try:
    import concourse.bass as bass
except ImportError:
    sys.path.insert(0, "/opt/trn_rl_repo")
    import concourse.bass as bass

import ml_dtypes
import concourse.tile as tile
from concourse import bacc
from concourse import mybir
from concourse.bass import ts
from concourse.bass_utils import run_bass_kernel_spmd

F32 = mybir.dt.float32
F32R = mybir.dt.float32r
BF16 = mybir.dt.bfloat16
ALU = mybir.AluOpType
ACTF = mybir.ActivationFunctionType
AX = mybir.AxisListType
NPBF = ml_dtypes.bfloat16

N_CORES = 8
D_IN = 4096
D_OUT = 4096
RANK = 16
NUM_EXPERTS = 8
ER = NUM_EXPERTS * RANK  # 128
TOP_K = 2
SCALING = 32.0 / RANK  # 2.0


def build_nc(T=1024, KT=32, OT=32):
    """Build the per-core Bass kernel. T tokens, KT k-tiles (d_in=128*KT),
    OT out-tiles (d_out=128*OT). T must be a multiple of 512."""
    TH = T // 512  # token halves for 512-wide matmuls
    TS = T // 128  # token subtiles for router/softmax
    nc = bacc.Bacc(None, target_bir_lowering=False, dynamic_dma_scratch_size=1024)

    xt = nc.dram_tensor("xt", [128, KT, T], F32R, kind="ExternalInput")
    wt = nc.dram_tensor("wt", [OT, 128, KT, 128], F32R, kind="ExternalInput")
    at = nc.dram_tensor("at", [128, KT, ER], F32R, kind="ExternalInput")
    bt = nc.dram_tensor("bt", [ER, 128 * OT], F32R, kind="ExternalInput")
    rt = nc.dram_tensor("rt", [128, KT, NUM_EXPERTS], F32R, kind="ExternalInput")
    bias2 = nc.dram_tensor("bias2", [128, OT], F32, kind="ExternalInput")
    id2 = nc.dram_tensor("id2", [128, 128], F32, kind="ExternalInput")
    expand = nc.dram_tensor("expand", [NUM_EXPERTS, ER], F32, kind="ExternalInput")
    outt = nc.dram_tensor("outt", [OT, 128, T], F32, kind="ExternalOutput")

    with tile.TileContext(nc) as tc, ExitStack() as ctx:
        const = ctx.enter_context(tc.tile_pool(name="const", bufs=1))
        xpool = ctx.enter_context(tc.tile_pool(name="xp", bufs=1))
        wpool = ctx.enter_context(tc.tile_pool(name="wp", bufs=4))
        btp = ctx.enter_context(tc.tile_pool(name="btp", bufs=2))
        hpool = ctx.enter_context(tc.tile_pool(name="hp", bufs=1))
        smt = ctx.enter_context(tc.tile_pool(name="smt", bufs=2))
        opool = ctx.enter_context(tc.tile_pool(name="op", bufs=2))
        pmain = ctx.enter_context(
            tc.tile_pool(name="pmain", bufs=max(3 * TH, 4), space="PSUM")
        )
        psmall = ctx.enter_context(tc.tile_pool(name="psm", bufs=2, space="PSUM"))
        E = NUM_EXPERTS

        # ---- window DMAs (sync HWDGE queue), front-loaded so kt=0 lands
        # fast; bf16 halves every transfer vs the fp32 version ----
        at_s = wpool.tile([128, KT, 128], F32R, tag="w")
        w0_s = wpool.tile([128, KT, 128], F32R, tag="w")
        rt_s = const.tile([128, KT, E], F32R)
        x_s = xpool.tile([128, KT, T], F32R)
        nsw = min(8, KT)
        wpc = KT // nsw

        id_s = bias_s = exp_s = b0_s = None

        def emit_consts():
            nonlocal id_s, bias_s, exp_s, b0_s
            id_s = const.tile([128, 128], F32)
            nc.sync.dma_start(id_s, id2[:])
            bias_s = const.tile([128, OT], F32)
            nc.sync.dma_start(bias_s, bias2[:])
            exp_s = const.tile([E, ER], F32)
            nc.sync.dma_start(exp_s, expand[:])
            b0_s = const.tile([ER, 128], F32R)
            nc.scalar.dma_start(b0_s, bt[:, 0:128])

        def xdma(kt, eng):
            if kt < 2 and KT >= 32:
                for th in range(TH):
                    eng.dma_start(x_s[:, kt, ts(th, 512)], xt[:, kt, ts(th, 512)])
            else:
                eng.dma_start(x_s[:, kt, :], xt[:, kt, :])

        # front-loaded window DMAs, interleaved so the first k-tiles land
        # fast (baseline-tuned pattern: th-split head, 1MB x chunks after)
        for q in range(nsw):
            if q == 0 and wpc > 1:
                nc.sync.dma_start(at_s[:, 0:1, :], at[:, 0:1, :])
                nc.sync.dma_start(rt_s, rt[:])
                nc.sync.dma_start(w0_s[:, 0:1, :], wt[0, :, 0:1, :])
                if KT >= 32:
                    for th in range(TH):
                        nc.sync.dma_start(
                            x_s[:, 0, ts(th, 512)], xt[:, 0, ts(th, 512)]
                        )
                nc.sync.dma_start(at_s[:, 1:wpc, :], at[:, 1:wpc, :])
                nc.sync.dma_start(w0_s[:, 1:wpc, :], wt[0, :, 1:wpc, :])
            else:
                nc.sync.dma_start(at_s[:, ts(q, wpc), :], at[:, ts(q, wpc), :])
                nc.sync.dma_start(w0_s[:, ts(q, wpc), :], wt[0, :, ts(q, wpc), :])
            if q == 0 or KT < 32:
                for kt in range(q * wpc, (q + 1) * wpc):
                    if q == 0 and kt == 0 and wpc > 1 and KT >= 32:
                        continue
                    xdma(kt, nc.sync)
            else:
                # 2-ktile (1MB) chunks: fewer descriptors, higher sustained rate
                for k0 in range(q * wpc, (q + 1) * wpc, 2):
                    nc.sync.dma_start(x_s[:, k0 : k0 + 2, :], xt[:, k0 : k0 + 2, :])
            if q == 5:
                emit_consts()
        if id_s is None:
            emit_consts()

        # ---- window: A-proj + router + base(ot=0) share the x stream ----
        ph = [pmain.tile([128, 512], F32, tag="pm", name=f"ph{i}") for i in range(TH)]
        plT = [pmain.tile([E, 512], F32, tag="pm", name=f"plT{i}") for i in range(TH)]
        po0 = [pmain.tile([128, 512], F32, tag="pm", name=f"po0{i}") for i in range(TH)]
        for kt in range(KT):
            st, sp = kt == 0, kt == KT - 1
            xcs = [x_s[:, kt, ts(th, 512)] for th in range(TH)]
            if kt < 2:
                # th-major: start on the first 512 tokens before the
                # second half of the kt tile has landed
                for th in range(TH):
                    nc.tensor.matmul(ph[th], at_s[:, kt, :], xcs[th], start=st, stop=sp)
                    nc.tensor.matmul(plT[th], rt_s[:, kt, :], xcs[th], start=st, stop=sp)
                    nc.tensor.matmul(po0[th], w0_s[:, kt, :], xcs[th], start=st, stop=False)
            else:
                for th in range(TH):
                    nc.tensor.matmul(ph[th], at_s[:, kt, :], xcs[th], start=st, stop=sp)
                for th in range(TH):
                    nc.tensor.matmul(plT[th], rt_s[:, kt, :], xcs[th], start=st, stop=sp)
                for th in range(TH):
                    nc.tensor.matmul(po0[th], w0_s[:, kt, :], xcs[th], start=st, stop=False)

        def load_w(ot):
            w_s = wpool.tile([128, KT, 128], F32R, tag="w")
            nsl = min(4, KT)
            for q in range(nsl):
                nc.sync.dma_start(
                    w_s[:, ts(q, KT // nsl), :], wt[ot, :, ts(q, KT // nsl), :]
                )
            b_sl = btp.tile([ER, 128], F32R)
            nc.sync.dma_start(b_sl, bt[:, ts(ot, 128)])
            return w_s, b_sl

        h_s = hpool.tile([128, T], F32R)  # A-proj, then weighted in place
        h_f = h_s.bitcast(F32)  # f32 view for DVE ops
        lT = hpool.tile([E, T], F32)
        # lT first: the ptl transposes interleaved into base(1) wait on it
        for th in range(TH):
            nc.vector.tensor_copy(lT[:, ts(th, 512)], plT[th])
        for th in range(TH):
            nc.vector.tensor_copy(h_s[:, ts(th, 512)], ph[th])

        # ---- softmax/top-2 chain, emitted piecewise between base matmuls ----
        ptl_all = psmall.tile([128, E * TS], F32, tag="ptl", bufs=1)
        cmb2s = [None] * TS
        e_ts = [None] * TS
        e12s = [None] * TS
        invs = [None] * TS
        cts = [None] * TS

        def sm_pre(s_i):
            # PE: transpose logits subtile to [128 tok, E]; all TS subtiles
            # share one PSUM bank (disjoint columns) to avoid ring waits
            nc.tensor.transpose(
                ptl_all[:, ts(s_i, E)], lT[:, ts(s_i, 128)], id_s[:E, :E]
            )

        def sm_dve_head(s_i):
            # DVE: maxes + top-2 mask; ACT: exps (no max-shift, |logit| < ~4)
            l = ptl_all[:, ts(s_i, E)]
            m12 = smt.tile([128, 2], F32, tag="m12", name=f"m12_{s_i}")
            nc.vector.reduce_max(m12[:, 0:1], l, axis=AX.X)
            pen = smt.tile([128, E], F32, tag="pen", name=f"pen{s_i}")
            nc.vector.tensor_scalar(
                pen, l, m12[:, 0:1], -1e30, op0=ALU.is_equal, op1=ALU.mult
            )
            msk = smt.tile([128, E], F32, tag="msk", name=f"msk{s_i}")
            nc.vector.tensor_tensor(msk, l, pen, op=ALU.add)
            nc.vector.reduce_max(m12[:, 1:2], msk, axis=AX.X)
            ge = smt.tile([128, E], F32, tag="ge", bufs=4, name=f"ge{s_i}")
            nc.vector.tensor_scalar(ge, l, m12[:, 1:2], None, op0=ALU.is_ge)
            e12 = smt.tile([128, 2], F32, tag="e12", bufs=4, name=f"e12_{s_i}")
            nc.scalar.activation(e12, m12, ACTF.Exp)
            e_t = smt.tile([128, E], F32, tag="e_t", bufs=4, name=f"e_t{s_i}")
            nc.scalar.activation(e_t, l, ACTF.Exp)
            e12s[s_i] = e12
            e_ts[s_i] = (e_t, ge)

        def sm_dve_tail(s_i):
            # DVE: renormalize (runs 2 hooks later so the ACT exps are done)
            e12 = e12s[s_i]
            den = smt.tile([128, 1], F32, tag="den", name=f"den{s_i}")
            nc.vector.tensor_tensor(den, e12[:, 0:1], e12[:, 1:2], op=ALU.add)
            inv = smt.tile([128, 1], F32, tag="inv", bufs=4, name=f"inv{s_i}")
            nc.vector.reciprocal(inv, den)
            e_t, ge = e_ts[s_i]
            cmb = smt.tile([128, E], F32, tag="cmb", name=f"cmb{s_i}")
            nc.vector.tensor_tensor(cmb, e_t, ge, op=ALU.mult)
            # all TS cmb2 tiles are alive until their (late) pt transpose;
            # fewer bufs deadlocks the DVE queue against the PE queue
            cmb2 = smt.tile([128, E], F32, tag="cmb2", bufs=TS, name=f"cmb2{s_i}")
            nc.vector.tensor_scalar(cmb2, cmb, inv, None, op0=ALU.mult)
            cmb2s[s_i] = cmb2

        def sm_pt(s_i):
            # PE: transpose combine back to [E, 128 tok]; DVE: copy out
            p = psmall.tile([E, 128], F32, tag="ps", bufs=1, name=f"pt{s_i}")
            nc.tensor.transpose(p, cmb2s[s_i], id_s)
            ct = smt.tile([E, 128], F32, tag="ct", name=f"ct{s_i}")
            nc.vector.tensor_copy(ct, p)
            cts[s_i] = ct

        def sm_pc(s_i):
            # PE: expand combine to [ER, 128] (x scaling); DVE: weight h
            p = psmall.tile([128, 128], F32, tag="ps", bufs=1, name=f"pc{s_i}")
            nc.tensor.matmul(p, exp_s, cts[s_i], start=True, stop=True)
            nc.vector.tensor_tensor(
                h_s[:, ts(s_i, 128)], h_f[:, ts(s_i, 128)], p, op=ALU.mult
            )

        def emit_base(ot, w_s, hooks=None):
            # kt outer / th inner: consecutive matmuls share the stationary
            # weight tile; optional per-kt hooks inject softmax-chain ops
            # into the PE stream so they hide under the matmuls
            pos = [
                pmain.tile([128, 512], F32, tag="pm", name=f"po_{ot}_{th}")
                for th in range(TH)
            ]
            for kt in range(KT):
                for th in range(TH):
                    nc.tensor.matmul(
                        pos[th],
                        w_s[:, kt, :],
                        x_s[:, kt, ts(th, 512)],
                        start=(kt == 0),
                        stop=False,
                    )
                if hooks is not None:
                    for fn in hooks.get(kt, ()):
                        fn()
            return pos

        def emit_tail(ot, pos, b_sl):
            for th in range(TH):
                nc.tensor.matmul(
                    pos[th], b_sl, h_s[:, ts(th, 512)], start=False, stop=True
                )
                o_t = opool.tile([128, 512], F32, tag="o_t", name=f"ot_{ot}_{th}")
                nc.scalar.activation(
                    o_t, pos[th], ACTF.Identity, bias=bias_s[:, ot : ot + 1]
                )
                nc.sync.dma_start(outt[ot, :, ts(th, 512)], o_t)

        # hook schedules: ptl transposes early (paced for the 2-slot psmall
        # ring), combine transposes/expands late; overflow into base(2)
        hooks1 = {}
        for s in range(TS):
            hooks1.setdefault(2 + 2 * s, []).append(
                lambda s=s: (sm_pre(s), sm_dve_head(s))
            )
            hooks1.setdefault(6 + 2 * s, []).append(lambda s=s: sm_dve_tail(s))
        hooks2 = {}
        for s in range(TS):
            hooks2.setdefault(2 * s, []).append(lambda s=s: sm_pt(s))
            hooks2.setdefault(2 * s + 1, []).append(lambda s=s: sm_pc(s))

        first = min(1, OT - 1)
        w1, b1 = load_w(first)
        w2, b2 = load_w(2)
        pos1 = emit_base(first, w1, hooks1)
        pos2 = emit_base(2, w2, hooks2)

        emit_tail(first, pos1, b1)

        # ---- ot=0 LoRA term accumulated into the held PSUM group ----
        for th in range(TH):
            nc.tensor.matmul(
                po0[th], b0_s, h_s[:, ts(th, 512)], start=False, stop=True
            )
            o_t = opool.tile([128, 512], F32, name=f"oo0_{th}", tag="o_t")
            nc.scalar.activation(o_t, po0[th], ACTF.Identity, bias=bias_s[:, 0:1])
            nc.sync.dma_start(outt[0, :, ts(th, 512)], o_t)

        # ---- remaining o-tiles; tail(ot-1) after base(ot) so the B matmul
        # never waits on the combine chain and output DMA stays 1-ot behind
        prev = (2, pos2, b2)
        for ot in range(3, OT):
            w_s, b_sl = load_w(ot)
            pos = emit_base(ot, w_s)
            emit_tail(*prev)
            prev = (ot, pos, b_sl)
        emit_tail(*prev)

    nc.compile()
    return nc


def prep_shared(W_base, b_base, W_router, A_stack, B_stack, KT=32, OT=32):
    """Host-side layout prep for the replicated weights (bf16)."""
    D = KT * 128
    O = OT * 128
    W_base = np.asarray(W_base, dtype=np.float32)
    wt = np.ascontiguousarray(
        W_base.reshape(OT, 128, KT, 128).transpose(0, 3, 2, 1)
    )
    A_all = np.asarray(A_stack, dtype=np.float32).reshape(ER, D)
    at = np.ascontiguousarray(A_all.reshape(ER, KT, 128).transpose(2, 1, 0))
    bt = np.ascontiguousarray(
        np.asarray(B_stack, dtype=np.float32).transpose(0, 2, 1).reshape(ER, O)
    )
    rtT = np.asarray(W_router, dtype=np.float32).T  # [D, E]
    rt = np.ascontiguousarray(rtT.reshape(KT, 128, NUM_EXPERTS).transpose(1, 0, 2))
    bias2 = np.ascontiguousarray(np.asarray(b_base, dtype=np.float32).reshape(OT, 128).T)
    id2 = np.eye(128, dtype=np.float32)
    expand = np.repeat(
        np.eye(NUM_EXPERTS, dtype=np.float32) * np.float32(SCALING), RANK, axis=1
    )
    return dict(wt=wt, at=at, bt=bt, rt=rt, bias2=bias2, id2=id2, expand=expand)


def make_in_maps(x, W_base, b_base, W_router, A_stack, B_stack, KT=32, OT=32):
    x = np.asarray(x, dtype=np.float32)
    xf = x.reshape(-1, D_IN)
    N = xf.shape[0]
    T = N // N_CORES
    shared = prep_shared(W_base, b_base, W_router, A_stack, B_stack, KT, OT)
    in_maps = []
    for c in range(N_CORES):
        x_c = xf[c * T : (c + 1) * T]  # [T, D]
        xtc = np.ascontiguousarray(x_c.reshape(T, KT, 128).transpose(2, 1, 0))
        m = dict(shared)
        m["xt"] = xtc
        in_maps.append(m)
    return in_maps


_NC_CACHE = {}


def _get_nc(T, KT, OT):
    key = (T, KT, OT)
    if key not in _NC_CACHE:
        _NC_CACHE[key] = build_nc(T, KT, OT)
    return _NC_CACHE[key]


def kernel(x, W_base, b_base, W_router, A_stack, B_stack):
    x = np.asarray(x, dtype=np.float32)
    orig_shape = x.shape
    N = x.reshape(-1, D_IN).shape[0]
    T = N // N_CORES
    KT = D_IN // 128
    OT = D_OUT // 128

    nc = _get_nc(T, KT, OT)
    in_maps = make_in_maps(x, W_base, b_base, W_router, A_stack, B_stack, KT, OT)

    res = run_bass_kernel_spmd(nc, in_maps, core_ids=list(range(N_CORES)))
    out = np.empty((N, D_OUT), dtype=np.float32)
    for c in range(N_CORES):
        outt = res.results[c]["outt"]  # [OT, 128, T]
        out[c * T : (c + 1) * T] = outt.transpose(2, 0, 1).reshape(T, D_OUT)
    return out.reshape(orig_shape[:-1] + (D_OUT,))
